# Initial kernel scaffold
#
"""Trainium2 Bass kernel for nn_CalibrationError (ECE/MCE over softmax confidences).

Contract: kernel(logits[N,C] f32, labels[N] int64) -> (ece, mce) f32 scalars,
matching reference.py. Internally shards rows across 8 NeuronCores, computes a
15-bin cumulative (count, sum_conf, sum_acc) histogram on-device per core, and
finishes the tiny ECE/MCE arithmetic on host.

Self-contained: hardcodes shapes/sharding; only imports the concourse toolchain.
"""

import sys

if "/opt/trn_rl_repo" not in sys.path:
    sys.path.insert(0, "/opt/trn_rl_repo")

import numpy as np

import concourse.bass as bass
import concourse.mybir as mybir
from concourse.tile import TileContext
from contextlib import ExitStack

# ---------------------------------------------------------------- constants
P = 128          # SBUF partitions
C = 100          # classes
R = 40           # rows per partition per tile
T = 49           # tiles per core
NCORES = 8
NBINS = 15
ROWS_PER_CORE = P * R * T          # 250_880 (incl. padding)
REAL_ROWS_PER_CORE = 2_000_000 // NCORES  # 250_000
PAD_LOGIT = -10000.0               # exp() underflows to exactly 0.0

f32 = mybir.dt.float32
i32 = mybir.dt.int32
Alu = mybir.AluOpType
Act = mybir.ActivationFunctionType

# Toggle: row-sums via PE identity-chunk matmuls (fast) vs DVE reduce (simple).
USE_PE_SUM = True


def build_nc(p=P, c=C, r=R, t=T):
    """Build the per-core Bass module (SPMD: same program on all cores)."""
    nc = bass.Bass()

    x = nc.declare_dram_parameter("x", [t * p * r, c], f32, isOutput=False)
    lab = nc.declare_dram_parameter("lab", [t, p, r], i32, isOutput=False)
    rev = nc.declare_dram_parameter("rev", [p, r * c], i32, isOutput=False)
    ident = nc.declare_dram_parameter("ident", [p, p], f32, isOutput=False)
    out = nc.declare_dram_parameter("out", [NBINS, 3], f32, isOutput=True)

    xv = x[:, :].rearrange("(t p r) c -> t p (r c)", t=t, p=p, r=r)

    with TileContext(nc) as tc, ExitStack() as ctx:
        consts = ctx.enter_context(tc.tile_pool(name="consts", bufs=1))
        work = ctx.enter_context(tc.tile_pool(name="work", bufs=2))
        small = ctx.enter_context(tc.tile_pool(name="small", bufs=3))
        psum = ctx.enter_context(tc.tile_pool(name="psum", bufs=2, space="PSUM"))

        rev_t = consts.tile([p, r * c], i32, tag="rev_t")
        nc.sync.dma_start(out=rev_t[:], in_=rev[:, :])
        ident_t = consts.tile([p, p], f32, tag="ident_t")
        nc.sync.dma_start(out=ident_t[:], in_=ident[:, :])
        mask_hi = consts.tile([p, 1], i32, tag="mask_hi")  # ~127
        nc.vector.memset(mask_hi[:], -128)
        mask_lo = consts.tile([p, 1], i32, tag="mask_lo")  # 127
        nc.vector.memset(mask_lo[:], 127)
        hist = consts.tile([NBINS, 3], f32, tag="hist")
        nc.vector.memset(hist[:], 0.0)

        for it in range(t):
            xt = work.tile([p, r * c], f32, tag="xt")
            nc.sync.dma_start(out=xt[:], in_=xv[it])
            labt = work.tile([p, r], i32, tag="labt")
            nc.sync.dma_start(out=labt[:], in_=lab[it, :, :])

            # e = exp(x)  (no max-subtraction needed: |x| < 90)
            et = work.tile([p, r * c], f32, tag="et")
            nc.scalar.activation(out=et[:], in_=xt[:], func=Act.Exp)
            e3 = et[:].rearrange("p (r c) -> p r c", r=r)

            # pack value+index: pk = (bits(e) & ~127) | (127 - class)
            pk = work.tile([p, r * c], i32, tag="pk")
            nc.vector.scalar_tensor_tensor(
                out=pk[:],
                in0=et[:].bitcast(i32),
                scalar=mask_hi[:],
                in1=rev_t[:],
                op0=Alu.bitwise_and,
                op1=Alu.bitwise_or,
            )
            # grouped argmax+max: pm[p, r] = max over classes (int compare ==
            # float compare for positive floats)
            pm = small.tile([p, r], i32, tag="pm")
            nc.vector.tensor_reduce(
                out=pm[:],
                in_=pk[:].rearrange("p (r c) -> p r c", r=r),
                axis=mybir.AxisListType.X,
                op=Alu.max,
            )

            # row sums s[p, r] = sum_c e[p, r, c]
            if USE_PE_SUM:
                ps = psum.tile([p, r], f32, tag="ps")
                for cc in range(c):
                    nc.tensor.matmul(
                        ps[:],
                        lhsT=ident_t[:],
                        rhs=e3[:, :, cc],
                        start=(cc == 0),
                        stop=(cc == c - 1),
                    )
                s_sb = small.tile([p, r], f32, tag="s_sb")
                nc.vector.tensor_copy(out=s_sb[:], in_=ps[:])
            else:
                s_sb = small.tile([p, r], f32, tag="s_sb")
                nc.vector.tensor_reduce(
                    out=s_sb[:], in_=e3, axis=mybir.AxisListType.X, op=Alu.add
                )

            # clamp away zeros (pad rows) then reciprocal
            nc.vector.tensor_scalar_max(s_sb[:], s_sb[:], 1e-30)
            rs = small.tile([p, r], f32, tag="rs")
            nc.vector.reciprocal(out=rs[:], in_=s_sb[:])

            # vals = [conf, acc, ones] laid out [p, 3, r]
            vals = small.tile([p, 3 * r], f32, tag="vals")
            v3 = vals[:].rearrange("p (k r) -> p k r", k=3)

            # me = float(pm & ~127); conf = me * (1/s)
            meb = small.tile([p, r], i32, tag="meb")
            nc.vector.tensor_single_scalar(
                out=meb[:], in_=pm[:], scalar=mask_hi[:], op=Alu.bitwise_and
            )
            nc.vector.tensor_tensor(
                out=v3[:, 0, :], in0=meb[:].bitcast(f32), in1=rs[:], op=Alu.mult
            )
            # acc = (pm & 127) == (127 - label)
            jrev = small.tile([p, r], i32, tag="jrev")
            nc.vector.tensor_single_scalar(
                out=jrev[:], in_=pm[:], scalar=mask_lo[:], op=Alu.bitwise_and
            )
            nc.vector.tensor_tensor(
                out=v3[:, 1, :], in0=jrev[:], in1=labt[:], op=Alu.is_equal
            )
            nc.vector.memset(v3[:, 2, :], 1.0)

            # ge[p, b, r] = conf > b/15   (strict: pad rows have conf == 0.0)
            ge = small.tile([p, NBINS * r], f32, tag="ge")
            g3 = ge[:].rearrange("p (b r) -> p b r", b=NBINS)
            for b in range(NBINS):
                nc.vector.tensor_single_scalar(
                    out=g3[:, b, :], in_=v3[:, 0, :], scalar=float(b) / NBINS,
                    op=Alu.is_gt,
                )

            # cumulative histogram: out[b, k] += sum_rows ge_b * vals_k
            ph = psum.tile([NBINS, 3], f32, tag="ph")
            for rr in range(r):
                nc.tensor.matmul(
                    ph[:],
                    lhsT=g3[:, :, rr],
                    rhs=v3[:, :, rr],
                    start=(rr == 0),
                    stop=(rr == r - 1),
                )
            nc.vector.tensor_tensor(
                out=hist[:], in0=hist[:], in1=ph[:], op=Alu.add
            )

        nc.sync.dma_start(out=out[:, :], in_=hist[:])

    return nc


# ---------------------------------------------------------------- host side

def _prep_core_inputs(logits, labels, core):
    """Build the per-core input dict (padded, tile-layout labels)."""
    lo = core * REAL_ROWS_PER_CORE
    hi = lo + REAL_ROWS_PER_CORE
    x = np.full((ROWS_PER_CORE, C), PAD_LOGIT, dtype=np.float32)
    x[: REAL_ROWS_PER_CORE] = logits[lo:hi]
    lab = np.zeros(ROWS_PER_CORE, dtype=np.int32)
    lab[: REAL_ROWS_PER_CORE] = labels[lo:hi].astype(np.int32)
    labrev = (127 - lab).reshape(T, P, R)
    return {"x": x, "lab": labrev}


def _shared_inputs():
    rev = np.broadcast_to(
        (127 - np.arange(C, dtype=np.int32))[None, None, :], (P, R, C)
    ).reshape(P, R * C).copy()
    ident = np.eye(P, dtype=np.float32)
    return {"rev": rev, "ident": ident}


def _finish(hists):
    """hists: list of [15, 3] cumulative-threshold sums -> (ece, mce)."""
    cum = np.zeros((NBINS + 1, 3), dtype=np.float64)
    for h in hists:
        cum[:NBINS] += h.astype(np.float64)
    per_bin = cum[:NBINS] - cum[1:]  # [15, 3]: sum_conf, sum_acc, count
    sum_conf, sum_acc, counts = per_bin[:, 0], per_bin[:, 1], per_bin[:, 2]
    nonempty = counts > 0
    safe = np.where(nonempty, counts, 1.0)
    gap = np.abs(sum_conf / safe - sum_acc / safe)
    n_total = float(2_000_000)
    ece = np.sum(np.where(nonempty, gap * counts / n_total, 0.0))
    mce = np.max(np.where(nonempty, gap, -np.inf)) if nonempty.any() else 1.0
    return np.float32(ece), np.float32(mce)


_NC_CACHE = {}


def kernel(logits, labels):
    from concourse.bass_utils import run_bass_kernel_spmd

    logits = np.asarray(logits, dtype=np.float32)
    labels = np.asarray(labels)

    if "nc" not in _NC_CACHE:
        _NC_CACHE["nc"] = build_nc()
    nc = _NC_CACHE["nc"]

    shared = _shared_inputs()
    in_maps = [
        {**_prep_core_inputs(logits, labels, core), **shared}
        for core in range(NCORES)
    ]
    res = run_bass_kernel_spmd(nc, in_maps, list(range(NCORES)))
    hists = [res.results[i]["out"] for i in range(NCORES)]
    return _finish(hists)


# revision 6
# speedup vs baseline: 41.6484x; 41.6484x over previous
"""Trainium2 Bass kernel for nn_CalibrationError (ECE/MCE over softmax confidences).

Contract: kernel(logits[N,C] f32, labels[N] int64) -> (ece, mce) f32 scalars,
matching reference.py. Internally shards rows across 8 NeuronCores, computes a
15-bin cumulative (count, sum_conf, sum_acc) histogram on-device per core, and
finishes the tiny ECE/MCE arithmetic on host.

Self-contained: hardcodes shapes/sharding; only imports the concourse toolchain.
"""

import sys

if "/opt/trn_rl_repo" not in sys.path:
    sys.path.insert(0, "/opt/trn_rl_repo")

import numpy as np

import concourse.bass as bass
import concourse.mybir as mybir
from concourse.tile import TileContext
from contextlib import ExitStack

# ---------------------------------------------------------------- constants
P = 128          # SBUF partitions
C = 100          # classes
R = 40           # rows per partition per tile
T = 49           # tiles per core
NCORES = 8
NBINS = 15
ROWS_PER_CORE = P * R * T          # 250_880 (incl. padding)
REAL_ROWS_PER_CORE = 2_000_000 // NCORES  # 250_000
PAD_LOGIT = -10000.0               # exp() underflows to exactly 0.0

f32 = mybir.dt.float32
i32 = mybir.dt.int32
Alu = mybir.AluOpType
Act = mybir.ActivationFunctionType

# Toggle: row-sums via PE identity-chunk matmuls (fast) vs DVE reduce (simple).
USE_PE_SUM = True


def build_nc(p=P, c=C, r=R, t=T):
    """Build the per-core Bass module (SPMD: same program on all cores)."""
    nc = bass.Bass()

    x = nc.declare_dram_parameter("x", [t * p * r, c], f32, isOutput=False)
    lab = nc.declare_dram_parameter("lab", [t, p, r], i32, isOutput=False)
    rev = nc.declare_dram_parameter("rev", [p, r * c], i32, isOutput=False)
    ident = nc.declare_dram_parameter("ident", [p, p], f32, isOutput=False)
    out = nc.declare_dram_parameter("out", [NBINS, 3], f32, isOutput=True)

    xv = x[:, :].rearrange("(t p r) c -> t p (r c)", t=t, p=p, r=r)

    with TileContext(nc) as tc, ExitStack() as ctx:
        consts = ctx.enter_context(tc.tile_pool(name="consts", bufs=1))
        work = ctx.enter_context(tc.tile_pool(name="work", bufs=2))
        small = ctx.enter_context(tc.tile_pool(name="small", bufs=3))
        psum = ctx.enter_context(tc.tile_pool(name="psum", bufs=2, space="PSUM"))

        rev_t = consts.tile([p, r * c], i32, tag="rev_t")
        nc.sync.dma_start(out=rev_t[:], in_=rev[:, :])
        ident_t = consts.tile([p, p], f32, tag="ident_t")
        nc.sync.dma_start(out=ident_t[:], in_=ident[:, :])
        mask_hi = consts.tile([p, 1], i32, tag="mask_hi")  # ~127
        nc.vector.memset(mask_hi[:], -128)
        mask_lo = consts.tile([p, 1], i32, tag="mask_lo")  # 127
        nc.vector.memset(mask_lo[:], 127)
        hist = consts.tile([NBINS, 3], f32, tag="hist")
        nc.vector.memset(hist[:], 0.0)
        zeros_i = consts.tile([p, r], i32, tag="zeros_i")
        nc.vector.memset(zeros_i[:], 0)

        for it in range(t):
            xt = work.tile([p, r * c], f32, tag="xt")
            nc.sync.dma_start(out=xt[:], in_=xv[it])
            labt = work.tile([p, r], i32, tag="labt")
            nc.sync.dma_start(out=labt[:], in_=lab[it, :, :])

            # e = exp(x)  (no max-subtraction needed: |x| < 90)
            et = work.tile([p, r * c], f32, tag="et")
            nc.scalar.activation(out=et[:], in_=xt[:], func=Act.Exp)
            e3 = et[:].rearrange("p (r c) -> p r c", r=r)

            # pack value+index: pk = (bits(e) & ~127) | (127 - class)
            pk = work.tile([p, r * c], i32, tag="pk")
            nc.vector.scalar_tensor_tensor(
                out=pk[:],
                in0=et[:].bitcast(i32),
                scalar=mask_hi[:],
                in1=rev_t[:],
                op0=Alu.bitwise_and,
                op1=Alu.bitwise_or,
            )
            # grouped argmax+max: pm[p, r] = max over classes. The DVE ALU is
            # fp32-internal, so reduce the packed bits AS float32: packed
            # values are positive normal floats, where fp32 ordering equals
            # bit ordering — the max is exact and index bits survive.
            pm = small.tile([p, r], f32, tag="pm")
            nc.vector.tensor_reduce(
                out=pm[:],
                in_=pk[:].bitcast(f32).rearrange("p (r c) -> p r c", r=r),
                axis=mybir.AxisListType.X,
                op=Alu.max,
            )

            # row sums s[p, r] = sum_c e[p, r, c]
            if USE_PE_SUM:
                ps = psum.tile([p, r], f32, tag="ps")
                for cc in range(c):
                    nc.tensor.matmul(
                        ps[:],
                        lhsT=ident_t[:],
                        rhs=e3[:, :, cc],
                        start=(cc == 0),
                        stop=(cc == c - 1),
                    )
                s_sb = small.tile([p, r], f32, tag="s_sb")
                nc.vector.tensor_copy(out=s_sb[:], in_=ps[:])
            else:
                s_sb = small.tile([p, r], f32, tag="s_sb")
                nc.vector.tensor_reduce(
                    out=s_sb[:], in_=e3, axis=mybir.AxisListType.X, op=Alu.add
                )

            # clamp away zeros (pad rows) then reciprocal
            nc.vector.tensor_scalar_max(s_sb[:], s_sb[:], 1e-30)
            rs = small.tile([p, r], f32, tag="rs")
            nc.vector.reciprocal(out=rs[:], in_=s_sb[:])

            # vals = [conf, acc, ones] laid out [p, 3, r]
            vals = small.tile([p, 3 * r], f32, tag="vals")
            v3 = vals[:].rearrange("p (k r) -> p k r", k=3)

            # me = float(pm & ~127); conf = me * (1/s)
            meb = small.tile([p, r], i32, tag="meb")
            nc.vector.scalar_tensor_tensor(
                out=meb[:], in0=pm[:].bitcast(i32), scalar=mask_hi[:],
                in1=zeros_i[:], op0=Alu.bitwise_and, op1=Alu.bitwise_or,
            )
            nc.vector.tensor_tensor(
                out=v3[:, 0, :], in0=meb[:].bitcast(f32), in1=rs[:], op=Alu.mult
            )
            # acc = (pm & 127) == (127 - label)
            jrev = small.tile([p, r], i32, tag="jrev")
            nc.vector.scalar_tensor_tensor(
                out=jrev[:], in0=pm[:].bitcast(i32), scalar=mask_lo[:],
                in1=zeros_i[:], op0=Alu.bitwise_and, op1=Alu.bitwise_or,
            )
            nc.vector.tensor_tensor(
                out=v3[:, 1, :], in0=jrev[:], in1=labt[:], op=Alu.is_equal
            )
            nc.vector.memset(v3[:, 2, :], 1.0)

            # ge[p, b, r] = conf > b/15   (strict: pad rows have conf == 0.0)
            ge = small.tile([p, NBINS * r], f32, tag="ge")
            g3 = ge[:].rearrange("p (b r) -> p b r", b=NBINS)
            for b in range(NBINS):
                nc.vector.tensor_single_scalar(
                    out=g3[:, b, :], in_=v3[:, 0, :], scalar=float(b) / NBINS,
                    op=Alu.is_gt,
                )

            # cumulative histogram: out[b, k] += sum_rows ge_b * vals_k
            ph = psum.tile([NBINS, 3], f32, tag="ph")
            for rr in range(r):
                nc.tensor.matmul(
                    ph[:],
                    lhsT=g3[:, :, rr],
                    rhs=v3[:, :, rr],
                    start=(rr == 0),
                    stop=(rr == r - 1),
                )
            nc.vector.tensor_tensor(
                out=hist[:], in0=hist[:], in1=ph[:], op=Alu.add
            )

        nc.sync.dma_start(out=out[:, :], in_=hist[:])

    return nc


# ---------------------------------------------------------------- host side

def _prep_core_inputs(logits, labels, core):
    """Build the per-core input dict (padded, tile-layout labels)."""
    lo = core * REAL_ROWS_PER_CORE
    hi = lo + REAL_ROWS_PER_CORE
    x = np.full((ROWS_PER_CORE, C), PAD_LOGIT, dtype=np.float32)
    x[: REAL_ROWS_PER_CORE] = logits[lo:hi]
    lab = np.zeros(ROWS_PER_CORE, dtype=np.int32)
    lab[: REAL_ROWS_PER_CORE] = labels[lo:hi].astype(np.int32)
    labrev = (127 - lab).reshape(T, P, R)
    return {"x": x, "lab": labrev}


def _shared_inputs():
    rev = np.broadcast_to(
        (127 - np.arange(C, dtype=np.int32))[None, None, :], (P, R, C)
    ).reshape(P, R * C).copy()
    ident = np.eye(P, dtype=np.float32)
    return {"rev": rev, "ident": ident}


def _finish(hists):
    """hists: list of [15, 3] cumulative-threshold sums -> (ece, mce)."""
    cum = np.zeros((NBINS + 1, 3), dtype=np.float64)
    for h in hists:
        cum[:NBINS] += h.astype(np.float64)
    per_bin = cum[:NBINS] - cum[1:]  # [15, 3]: sum_conf, sum_acc, count
    sum_conf, sum_acc, counts = per_bin[:, 0], per_bin[:, 1], per_bin[:, 2]
    nonempty = counts > 0
    safe = np.where(nonempty, counts, 1.0)
    gap = np.abs(sum_conf / safe - sum_acc / safe)
    n_total = float(2_000_000)
    ece = np.sum(np.where(nonempty, gap * counts / n_total, 0.0))
    mce = np.max(np.where(nonempty, gap, -np.inf)) if nonempty.any() else 1.0
    return np.float32(ece), np.float32(mce)


_NC_CACHE = {}


def kernel(logits, labels):
    from concourse.bass_utils import run_bass_kernel_spmd

    logits = np.asarray(logits, dtype=np.float32)
    labels = np.asarray(labels)

    if "nc" not in _NC_CACHE:
        _NC_CACHE["nc"] = build_nc()
    nc = _NC_CACHE["nc"]

    shared = _shared_inputs()
    in_maps = [
        {**_prep_core_inputs(logits, labels, core), **shared}
        for core in range(NCORES)
    ]
    res = run_bass_kernel_spmd(nc, in_maps, list(range(NCORES)))
    hists = [res.results[i]["out"] for i in range(NCORES)]
    return _finish(hists)
